# revision 35
# baseline (speedup 1.0000x reference)
"""Trainium2 Bass kernel for a dense transformer encoder layer.

Problem: B=1, S=4096, D=512, F=2048, H=8 heads (Dh=64), fp32 reference,
attention WITHOUT 1/sqrt(Dh) scaling, int mask (0 -> -1e9 before softmax),
two LayerNorms, ReLU FFN.

Sharding (query/row-parallel, no collectives): every core receives the
full transposed input xT = x.T and redundantly computes the full
kT = (x@wk).T and v = x@wv (cheaper than an AllGather at ~62 GB/s), plus
its own 512-query shard (xsT fp32 for the residual, xs16 fp16 for the q
projection) and its transposed bf16 {0,1} mask shard. Each core computes
attention + output projection + LN + FFN + LN for its queries and writes
outT (D, 512); the host transposes and concatenates the shards.

Dataflow is fully transposed (feature dims on SBUF partitions), so there
are no on-device transposes:
  scoresT[t,s] = sum_dh kT[dh,t] qT[dh,s]   two heads (K=64 each) packed
                                            into the PE via row groups,
                                            concurrent matmul pairs
  A^T = exp(scoresT) * maskT                ACT exp PSUM->SBUF bf16, DVE
                                            bf16 2x multiplies
  outT[dh,s], sums[s] = [v | 1]^T @ A^T     ones column yields softmax
                                            denominators for free
Softmax skips max-subtraction (|scores| < ~60, exp fits fp32/bf16 range);
the per-query 1/sum is folded in after the V-aggregation via a K=1
broadcast matmul. LayerNorm runs transposed: partition-dim statistics via
ones-vector matmuls, per-column stats broadcast with K=1 outer products,
gamma/beta folded into the broadcast (g x rstd, g x (-mu rstd) + be x 1).

dtypes: fp16 (10-bit mantissa = tf32-grade) for the QKV projections,
K/Q storage, scores, and FFN/out-projection weights+activations - fp16
enables fast weight loads (FWL + background weight buffer) which keeps
the PE array dense enough that the HAM clock monitor holds 2.4 GHz;
fp32r (fp32 storage, full-rate PE mode) for K=1 broadcast matmuls and
LN/residual arithmetic; bf16 for exp outputs / V / mask (exp needs the
bf16 exponent range); fp32 accumulation everywhere (PSUM).

Measured on 8 axon-tunneled trn2 cores: ~345 us, rel_l2 ~1.6e-3.
"""

import os

import numpy as np
import ml_dtypes

import concourse.bass as bass
import concourse.bacc as bacc
import concourse.tile as tile
from concourse import mybir
from concourse.bass import ts, ds
from concourse.bass_utils import run_bass_kernel_spmd

AF = mybir.ActivationFunctionType
F32 = mybir.dt.float32
DT = mybir.dt.float32r  # fp32 storage, single-pass PE mode (full rate at N>=256)
DT16 = mybir.dt.float16  # 10-bit mantissa: tf32-grade scores, FWL weight loads
BF16 = mybir.dt.bfloat16

N_CORES = 8
EPS = 1e-5
PACK_SCORES = os.environ.get("PACK_SCORES", "1") == "1"


def build_encoder_kernel(nc, S=4096, D=512, F=2048, H=8, n_cores=8,
                         pack_scores=PACK_SCORES):
    """Emit the SPMD per-core program. Returns nothing (declares DRAM I/O)."""
    P = 128
    SH = S // n_cores          # query shard per core
    DC = D // P                # feature chunks of 128
    FC = F // P                # ffn chunks of 128
    TB = S // 512              # 512-wide t blocks (phase 1)
    TC = S // P                # 128-tall t chunks (phase 2)
    Dh = D // H
    assert Dh == 64 and DC * P == D and SH % 2 == 0

    d = lambda name, shape, dt: nc.dram_tensor(name, shape, dt, kind="ExternalInput").ap()
    xT = d("xT", [D, S], DT16)
    xsT = d("xsT", [D, SH], DT)
    xs16 = d("xs16", [D, SH], DT16)
    maskT = d("maskT", [S, SH], BF16)
    wq, wk, wv, wo = (d(n, [D, D], DT16) for n in ("wq", "wk", "wv", "wo"))
    w1 = d("w1", [D, F], DT16)
    w2 = d("w2", [F, D], DT16)
    bq, bk, bo = (d(n, [D], F32) for n in ("bq", "bk", "bo"))
    bv = d("bv", [D], DT)
    b1 = d("b1", [F], F32)
    b2 = d("b2", [D], F32)
    g1, be1, g2, be2 = (d(n, [D], DT) for n in ("g1", "be1", "g2", "be2"))
    ones = d("ones", [512], DT)
    outT = nc.dram_tensor("outT", [D, SH], F32, kind="ExternalOutput").ap()

    with tile.TileContext(nc) as tc:
        _emit(nc, tc, locals())


def _emit(nc, tc, io):
    P = 128
    xT, xsT, maskT = io["xT"], io["xsT"], io["maskT"]
    outT = io["outT"]
    S, D, F, H = io["S"], io["D"], io["F"], io["H"]
    SH, DC, FC, TB, TC, Dh = io["SH"], io["DC"], io["FC"], io["TB"], io["TC"], io["Dh"]
    pack_scores = io["pack_scores"]
    HPC = P // Dh              # heads per 128-feature chunk (2)

    from contextlib import ExitStack
    with ExitStack() as root:
        # ---- global pools (live for the whole kernel) ----
        gconst = root.enter_context(tc.tile_pool(name="gconst", bufs=1))
        gbig = root.enter_context(tc.tile_pool(name="gbig", bufs=1))

        ones_row = gconst.tile([1, P], DT)        # lhsT for K=1 broadcasts
        nc.sync.dma_start(out=ones_row, in_=io["ones"][None, :P])
        ones_col = gconst.tile([P, 1], DT)        # lhsT for partition sums
        nc.sync.dma_start(out=ones_col, in_=io["ones"][:P, None])
        eps_sb = gconst.tile([1, 1], F32)         # LN epsilon as a bias AP
        nc.vector.memset(eps_sb, EPS)



        xs_sb = gbig.tile([P, DC, SH], DT)        # own x shard, transposed (residual)
        nc.sync.dma_start(out=xs_sb, in_=xsT.rearrange("(c p) s -> p c s", p=P))
        xs16_sb = gbig.tile([P, DC, SH], DT16)    # fp16 twin for the q projection
        nc.sync.dma_start(out=xs16_sb, in_=io["xs16"].rearrange("(c p) s -> p c s", p=P))

        def load_vec(name, chunks):               # (n,) -> [128, chunks]
            t = gconst.tile([P, chunks], F32, tag=f"vec_{name}", name=f"vec_{name}")
            nc.sync.dma_start(out=t, in_=io[name].rearrange("(c p) -> p c", p=P))
            return t

        bq_sb, bk_sb, bo_sb = (load_vec(n, DC) for n in ("bq", "bk", "bo"))
        bv_row = gconst.tile([1, D], DT)          # rhs for the +bv outer product
        nc.sync.dma_start(out=bv_row, in_=io["bv"][None, :])
        b1_sb = load_vec("b1", FC)
        b2_sb = load_vec("b2", DC)
        def load_row(name):                       # (n,) -> [1, n] f32r row
            t = gconst.tile([1, D], DT, tag=f"row_{name}", name=f"row_{name}")
            nc.sync.dma_start(out=t, in_=io[name][None, :])
            return t

        g1_row, be1_row, g2_row, be2_row = (load_row(n) for n in ("g1", "be1", "g2", "be2"))
        ones_s = gconst.tile([1, SH], DT)
        nc.sync.dma_start(out=ones_s, in_=io["ones"][None, :SH])

        # PE warmup: dummy matmuls on the first-arriving input keep the HAM
        # activity monitor busy so real matmuls start at 2.4 GHz
        with tc.tile_pool(name="warmps", bufs=1, space="PSUM") as warmps:
            wps = warmps.tile([1, SH], F32)
            for _ in range(8):
                nc.tensor.matmul(wps, lhsT=ones_col, rhs=xs_sb[:, 0, :],
                                 start=True, stop=True)

        attn_sb = gbig.tile([P, DC, SH], DT16)    # normalized attention output^T

        # weight-stream pool opened early so phase-3 DMAs can prefetch
        p3w = root.enter_context(tc.tile_pool(name="p3w", bufs=4))

        # =========== phase 1 then attention (GH=4, pair-batched) ===========
        with tc.tile_pool(name="attn_big", bufs=1) as abig:
            kT_sb = abig.tile([P, DC, S], DT16)       # (x@wk)^T, full sequence
            qT_sb = abig.tile([P, DC, SH], DT16)      # (xs@wq)^T
            v_sb = abig.tile([P, TC, H, Dh + 1], BF16)  # v chunks + ones column
            nc.vector.memset(v_sb[:, :, :, Dh:Dh + 1], 1.0)

            # ---------------- phase 1: qT, kT, v projections ----------------
            with tc.tile_pool(name="p1w", bufs=1) as p1w, \
                 tc.tile_pool(name="p1x", bufs=6) as p1x, \
                 tc.tile_pool(name="p1ps", bufs=8, space="PSUM") as p1ps:

                def load_w(name):                 # (D, n) -> [128, DC, n]
                    w = io[name]
                    t = p1w.tile([P, DC, w.shape[1]], DT16, tag=f"w_{name}", name=f"w_{name}")
                    nc.sync.dma_start(out=t, in_=w.rearrange("(c p) n -> p c n", p=P))
                    return t

                # first t-block of xT starts transferring before anything else
                xt0 = []
                for ci in range(DC):
                    t = p1x.tile([P, 512], DT16, tag=f"xt{ci}", name=f"xt0_{ci}")
                    nc.sync.dma_start(out=t, in_=xT[ds(ci * P, P), ts(0, 512)])
                    xt0.append(t)

                wq_sb, wk_sb, wv_sb = load_w("wq"), load_w("wk"), load_w("wv")

                # broadcast bv across partitions once: bvb[p, do] = bv[do]
                bv_ps = p1ps.tile([P, D], F32, tag="ps")
                nc.tensor.matmul(bv_ps, lhsT=ones_row[:, :P], rhs=bv_row,
                                 start=True, stop=True)
                bvb_sb = p1w.tile([P, D], F32, tag="bvb")
                nc.vector.tensor_copy(bvb_sb, bv_ps)

                # qT[do, s-shard] first
                for co in range(DC):
                    ps = p1ps.tile([P, SH], F32, tag="ps")
                    for ci in range(DC):
                        nc.tensor.matmul(ps, lhsT=wq_sb[:, ci, ds(co * P, P)],
                                         rhs=xs16_sb[:, ci, :], start=(ci == 0), stop=(ci == DC - 1))
                    nc.scalar.activation(out=qT_sb[:, co, :], in_=ps,
                                         func=AF.Identity, bias=bq_sb[:, co:co + 1])

                for tb in range(TB):              # 512-wide t blocks
                    if tb == 0:
                        xt = xt0
                    else:
                        xt = []
                        for ci in range(DC):
                            t = p1x.tile([P, 512], DT16, tag=f"xt{ci}")
                            nc.sync.dma_start(out=t, in_=xT[ds(ci * P, P), ts(tb, 512)])
                            xt.append(t)
                    # kT[do, t-block]
                    for co in range(DC):
                        ps = p1ps.tile([P, 512], F32, tag="ps")
                        for ci in range(DC):
                            nc.tensor.matmul(ps, lhsT=wk_sb[:, ci, ds(co * P, P)],
                                             rhs=xt[ci], start=(ci == 0), stop=(ci == DC - 1))
                        nc.scalar.activation(out=kT_sb[:, co, ts(tb, 512)], in_=ps,
                                             func=AF.Identity, bias=bk_sb[:, co:co + 1])
                    # v[t-block, :] (natural layout) + bv
                    for tj in range(4):
                        ps = p1ps.tile([P, D], F32, tag="ps")
                        for ci in range(DC):
                            nc.tensor.matmul(ps, lhsT=xt[ci][:, ds(tj * P, P)],
                                             rhs=wv_sb[:, ci, :], start=(ci == 0), stop=(ci == DC - 1))
                        nc.vector.tensor_add(
                            out=v_sb[:, tb * 4 + tj, :, 0:Dh],
                            in0=ps.rearrange("p (h d) -> p h d", h=H),
                            in1=bvb_sb.rearrange("p (h d) -> p h d", h=H))

            # ---------------- phase 2: attention ----------------
            GH = 4 if H >= 4 else H
            NPR = GH // HPC
            with tc.tile_pool(name="p2", bufs=3) as p2, \
                 tc.tile_pool(name="p2a", bufs=4) as p2a, \
                 tc.tile_pool(name="p2ps", bufs=1, space="PSUM") as p2ps:
                for g in range(H // GH):
                    heads = list(range(g * GH, (g + 1) * GH))
                    out_ps = {h: p2ps.tile([Dh + 1, SH], F32, tag=f"out{j}", name=f"out_ps{j}")
                              for j, h in enumerate(heads)}
                    for ti in range(TC):
                        m_t = p2.tile([P, SH], BF16, tag="mask")
                        nc.sync.dma_start(out=m_t, in_=maskT[ds(ti * P, P), :])
                        sc = [p2ps.tile([P, HPC, SH], F32, tag=f"sc{pr}", name=f"sc{pr}")
                              for pr in range(NPR)]
                        for j, h in enumerate(heads):
                            pr, half = j // HPC, j % HPC
                            c = h // HPC
                            pslice = ds(half * Dh, Dh)
                            nc.tensor.matmul(
                                sc[pr][:, half, :], lhsT=kT_sb[pslice, c, ds(ti * P, P)],
                                rhs=qT_sb[pslice, c, :], start=True, stop=True,
                                tile_position=(half * Dh, 0) if pack_scores else None)
                        for pr in range(NPR):
                            a_t = p2a.tile([P, HPC, SH], BF16, tag=f"a{pr}", name=f"a{pr}")
                            nc.scalar.activation(out=a_t, in_=sc[pr], func=AF.Exp)
                            am_t = p2a.tile([P, HPC, SH], BF16, tag=f"am{pr}", name=f"am{pr}")
                            for half in range(HPC):
                                nc.vector.tensor_mul(am_t[:, half, :], a_t[:, half, :], m_t)
                            for half in range(HPC):
                                j = pr * HPC + half
                                h = heads[j]
                                nc.tensor.matmul(out_ps[h], lhsT=v_sb[:, ti, h, :],
                                                 rhs=am_t[:, half, :],
                                                 start=(ti == 0), stop=(ti == TC - 1))
                    for j, h in enumerate(heads):
                        c, half = h // HPC, h % HPC
                        rec = p2.tile([1, SH], DT, tag="rec")
                        with nc.allow_low_precision(reason="fp32 storage"):
                            nc.vector.reciprocal(rec, out_ps[h][Dh:Dh + 1, :])
                        bc = p2ps.tile([Dh, SH], F32, tag=f"sc{j // HPC}", name=f"bc{j}")
                        nc.tensor.matmul(bc, lhsT=ones_row[:1, :Dh], rhs=rec,
                                         start=True, stop=True)
                        bc_sb = p2.tile([Dh, SH], DT, tag="bcsb")
                        nc.scalar.copy(bc_sb, bc)
                        nc.vector.tensor_mul(attn_sb[ds(half * Dh, Dh), c, :],
                                             out_ps[h][0:Dh, :], bc_sb)

        # ---------------- phase 3: out proj + LN1 + FFN + LN2 ----------------
        with tc.tile_pool(name="p3", bufs=2) as p3, \
             tc.tile_pool(name="p3big", bufs=1) as p3big, \
             tc.tile_pool(name="p3ps", bufs=2, space="PSUM") as p3ps, \
             tc.tile_pool(name="p3st", bufs=1, space="PSUM") as p3st, \
             tc.tile_pool(name="p3bc", bufs=2, space="PSUM") as p3bc:

            def layernorm(src, g_row, be_row, dst):
                """src/dst: lists of DC [128, SH] tiles; stats over partitions."""
                mu_ps = p3st.tile([1, SH], F32, tag="mu")
                m2_ps = p3st.tile([1, SH], F32, tag="m2")
                for c in range(DC):
                    nc.tensor.matmul(mu_ps, lhsT=ones_col, rhs=src[c],
                                     start=(c == 0), stop=(c == DC - 1))
                for c in range(DC):
                    sq = p3.tile([P, SH], DT, tag="sq")
                    nc.scalar.activation(out=sq, in_=src[c], func=AF.Square)
                    nc.tensor.matmul(m2_ps, lhsT=ones_col, rhs=sq,
                                     start=(c == 0), stop=(c == DC - 1))
                mu_s = p3.tile([1, SH], DT, tag="mu_s")
                m2_s = p3.tile([1, SH], DT, tag="m2_s")
                nc.vector.tensor_scalar_mul(mu_s, mu_ps, -1.0 / D)  # negated mean
                nc.vector.tensor_scalar_mul(m2_s, m2_ps, 1.0 / D)
                var_s = p3.tile([1, SH], DT, tag="var_s")
                nc.vector.tensor_mul(var_s, mu_s, mu_s)
                nc.vector.tensor_sub(var_s, m2_s, var_s)
                # rstd = (var+eps)^-1/2 = exp(-0.5 ln(var+eps)) on ACT
                rstd_s = p3.tile([1, SH], DT, tag="rstd_s")
                sd_s = p3.tile([1, SH], DT, tag="sd_s")
                nc.scalar.activation(out=sd_s, in_=var_s, func=AF.Sqrt, bias=eps_sb)
                with nc.allow_low_precision(reason="fp32 storage"):
                    nc.vector.reciprocal(rstd_s, sd_s)
                # offset row: -mu*rstd
                off_s = p3.tile([1, SH], DT, tag="off_s")
                nc.vector.tensor_mul(off_s, mu_s, rstd_s)
                # per-chunk fused affine: dst = x*(g x rstd) + (g x (-mu rstd) + be x 1)
                for c in range(DC):
                    sc_b = p3bc.tile([P, SH], F32, tag="sc_b")
                    of_b = p3bc.tile([P, SH], F32, tag="of_b")
                    nc.tensor.matmul(sc_b, lhsT=g_row[:, ds(c * P, P)], rhs=rstd_s,
                                     start=True, stop=True)
                    nc.tensor.matmul(of_b, lhsT=g_row[:, ds(c * P, P)], rhs=off_s,
                                     start=True, stop=False)
                    nc.tensor.matmul(of_b, lhsT=be_row[:, ds(c * P, P)], rhs=ones_s,
                                     start=False, stop=True)
                    t = p3.tile([P, SH], DT, tag="lnt")
                    nc.vector.tensor_mul(t, src[c], sc_b)
                    nc.vector.tensor_add(dst[c], t, of_b)

            # out projection + residual -> xr
            wo_v = io["wo"].rearrange("(c p) n -> p c n", p=P)
            xr = [p3big.tile([P, SH], DT, tag=f"xr{c}", name=f"xr{c}") for c in range(DC)]
            for co in range(DC):
                ps = p3ps.tile([P, SH], F32, tag="ps")
                wt = p3w.tile([P, DC, P], DT16, tag="wt")
                nc.sync.dma_start(out=wt, in_=wo_v[:, :, ds(co * P, P)])
                for ci in range(DC):
                    nc.tensor.matmul(ps, lhsT=wt[:, ci, :], rhs=attn_sb[:, ci, :],
                                     start=(ci == 0), stop=(ci == DC - 1))
                t = p3.tile([P, SH], DT, tag="res")
                nc.scalar.activation(out=t, in_=ps, func=AF.Identity, bias=bo_sb[:, co:co + 1])
                nc.vector.tensor_add(xr[co], t, xs_sb[:, co, :])

            x1 = [p3big.tile([P, SH], DT, tag=f"x1{c}", name=f"x1{c}") for c in range(DC)]
            layernorm(xr, g1_row, be1_row, x1)
            x1h = [p3big.tile([P, SH], DT16, tag=f"x1h{c}", name=f"x1h{c}") for c in range(DC)]
            for c in range(DC):
                nc.scalar.copy(x1h[c], x1[c])

            # FFN
            w1_v = io["w1"].rearrange("(c p) n -> p c n", p=P)
            w2_v = io["w2"].rearrange("(c p) n -> p c n", p=P)
            hT = p3big.tile([P, FC, SH], DT16, tag="hT")
            for fc in range(FC):
                ps = p3ps.tile([P, SH], F32, tag="ps")
                wt = p3w.tile([P, DC, P], DT16, tag="wt")
                nc.sync.dma_start(out=wt, in_=w1_v[:, :, ds(fc * P, P)])
                for ci in range(DC):
                    nc.tensor.matmul(ps, lhsT=wt[:, ci, :], rhs=x1h[ci],
                                     start=(ci == 0), stop=(ci == DC - 1))
                nc.scalar.activation(out=hT[:, fc, :], in_=ps, func=AF.Relu,
                                     bias=b1_sb[:, fc:fc + 1])
            xr2 = [p3big.tile([P, SH], DT, tag=f"xr2{c}", name=f"xr2{c}") for c in range(DC)]
            for co in range(DC):
                ps = p3ps.tile([P, SH], F32, tag="ps")
                wt2 = p3w.tile([P, FC, P], DT16, tag="wt2")
                nc.sync.dma_start(out=wt2, in_=w2_v[:, :, ds(co * P, P)])
                for fc in range(FC):
                    nc.tensor.matmul(ps, lhsT=wt2[:, fc, :], rhs=hT[:, fc, :],
                                     start=(fc == 0), stop=(fc == FC - 1))
                t = p3.tile([P, SH], DT, tag="res")
                nc.scalar.activation(out=t, in_=ps, func=AF.Identity, bias=b2_sb[:, co:co + 1])
                nc.vector.tensor_add(xr2[co], t, x1[co])

            x2 = [p3big.tile([P, SH], F32, tag=f"x2{c}", name=f"x2{c}") for c in range(DC)]
            layernorm(xr2, g2_row, be2_row, x2)
            for c in range(DC):
                nc.sync.dma_start(out=outT[ds(c * P, P), :], in_=x2[c])


# ---------------------------------------------------------------------------
# host-side entry point
# ---------------------------------------------------------------------------

_CACHE = {}


def _get_compiled(S, D, F, H):
    key = (S, D, F, H)
    if key not in _CACHE:
        nc = bacc.Bacc("TRN2", target_bir_lowering=False, debug=False,
                       num_devices=N_CORES)
        build_encoder_kernel(nc, S=S, D=D, F=F, H=H, n_cores=N_CORES)
        nc.compile()
        _CACHE[key] = nc
    return _CACHE[key]


def make_in_maps(x, mask, weights, S, D, n_cores=N_CORES):
    """Shard + lay out inputs per core. x: (S, D) f32; mask: (S, S) int."""
    SH = S // n_cores
    xT = np.ascontiguousarray(x.T)                       # (D, S)
    maskb = (mask != 0)
    in_maps = []
    for c in range(n_cores):
        sl = slice(c * SH, (c + 1) * SH)
        im = {
            "xT": xT.astype(np.float16),
            "xsT": np.ascontiguousarray(xT[:, sl]),
            "xs16": np.ascontiguousarray(xT[:, sl]).astype(np.float16),
            "maskT": np.ascontiguousarray(maskb[sl, :].T).astype(ml_dtypes.bfloat16),
            "ones": np.ones(512, np.float32),
        }
        im.update({k: (v.astype(np.float16) if k in ("wq", "wk", "wv", "wo", "w1", "w2")
                       else v) for k, v in weights.items()})
        in_maps.append(im)
    return in_maps


def kernel(**inputs):
    x = np.asarray(inputs["x"], np.float32)
    mask = np.asarray(inputs["mask"])
    B, S, D = x.shape
    F = inputs["w1"].shape[1]
    H = 8
    assert B == 1
    weights = {k: np.asarray(inputs[k], np.float32)
               for k in ("wq", "wk", "wv", "wo", "w1", "w2",
                         "bq", "bk", "bv", "bo", "b1", "b2",
                         "g1", "be1", "g2", "be2")}
    nc = _get_compiled(S, D, F, H)
    in_maps = make_in_maps(x[0], mask, weights, S, D)
    res = run_bass_kernel_spmd(nc, in_maps, list(range(N_CORES)))
    SH = S // N_CORES
    out = np.empty((S, D), np.float32)
    for c in range(N_CORES):
        out[c * SH:(c + 1) * SH, :] = res.results[c]["outT"].T
    return out[None]


# revision 37
# speedup vs baseline: 1.0044x; 1.0044x over previous
"""Trainium2 Bass kernel for a dense transformer encoder layer.

Problem: B=1, S=4096, D=512, F=2048, H=8 heads (Dh=64), fp32 reference,
attention WITHOUT 1/sqrt(Dh) scaling, int mask (0 -> -1e9 before softmax),
two LayerNorms, ReLU FFN.

Sharding (query/row-parallel, no collectives): every core receives the
full transposed input xT = x.T and redundantly computes the full
kT = (x@wk).T and v = x@wv (cheaper than an AllGather at ~62 GB/s), plus
its own 512-query shard (xsT fp32 for the residual, xs16 fp16 for the q
projection) and its transposed bf16 {0,1} mask shard. Each core computes
attention + output projection + LN + FFN + LN for its queries and writes
outT (D, 512); the host transposes and concatenates the shards.

Dataflow is fully transposed (feature dims on SBUF partitions), so there
are no on-device transposes:
  scoresT[t,s] = sum_dh kT[dh,t] qT[dh,s]   two heads (K=64 each) packed
                                            into the PE via row groups,
                                            concurrent matmul pairs
  A^T = exp(scoresT) * maskT                ACT exp PSUM->SBUF bf16, DVE
                                            bf16 2x multiplies
  outT[dh,s], sums[s] = [v | 1]^T @ A^T     ones column yields softmax
                                            denominators for free
Softmax skips max-subtraction (|scores| < ~60, exp fits fp32/bf16 range);
the per-query 1/sum is folded in after the V-aggregation via a K=1
broadcast matmul. LayerNorm runs transposed: partition-dim statistics via
ones-vector matmuls, per-column stats broadcast with K=1 outer products,
gamma/beta folded into the broadcast (g x rstd, g x (-mu rstd) + be x 1).

dtypes: fp16 (10-bit mantissa = tf32-grade) for the QKV projections,
K/Q storage, scores, and FFN/out-projection weights+activations - fp16
enables fast weight loads (FWL + background weight buffer) which keeps
the PE array dense enough that the HAM clock monitor holds 2.4 GHz;
fp32r (fp32 storage, full-rate PE mode) for K=1 broadcast matmuls and
LN/residual arithmetic; bf16 for exp outputs / V / mask (exp needs the
bf16 exponent range); fp32 accumulation everywhere (PSUM).

Measured on 8 axon-tunneled trn2 cores: ~345 us, rel_l2 ~1.6e-3.
"""

import os

import numpy as np
import ml_dtypes

import concourse.bass as bass
import concourse.bacc as bacc
import concourse.tile as tile
from concourse import mybir
from concourse.bass import ts, ds
from concourse.bass_utils import run_bass_kernel_spmd

AF = mybir.ActivationFunctionType
F32 = mybir.dt.float32
DT = mybir.dt.float32r  # fp32 storage, single-pass PE mode (full rate at N>=256)
DT16 = mybir.dt.float16  # 10-bit mantissa: tf32-grade scores, FWL weight loads
BF16 = mybir.dt.bfloat16

N_CORES = 8
EPS = 1e-5
PACK_SCORES = os.environ.get("PACK_SCORES", "1") == "1"


def build_encoder_kernel(nc, S=4096, D=512, F=2048, H=8, n_cores=8,
                         pack_scores=PACK_SCORES):
    """Emit the SPMD per-core program. Returns nothing (declares DRAM I/O)."""
    P = 128
    SH = S // n_cores          # query shard per core
    DC = D // P                # feature chunks of 128
    FC = F // P                # ffn chunks of 128
    TB = S // 512              # 512-wide t blocks (phase 1)
    TC = S // P                # 128-tall t chunks (phase 2)
    Dh = D // H
    assert Dh == 64 and DC * P == D and SH % 2 == 0

    d = lambda name, shape, dt: nc.dram_tensor(name, shape, dt, kind="ExternalInput").ap()
    xT = d("xT", [D, S], DT16)
    xsT = d("xsT", [D, SH], DT)
    xs16 = d("xs16", [D, SH], DT16)
    maskT = d("maskT", [S, SH], BF16)
    wq, wk, wv, wo = (d(n, [D, D], DT16) for n in ("wq", "wk", "wv", "wo"))
    w1 = d("w1", [D, F], DT16)
    w2 = d("w2", [F, D], DT16)
    bq, bk, bo = (d(n, [D], F32) for n in ("bq", "bk", "bo"))
    bv = d("bv", [D], DT)
    b1 = d("b1", [F], F32)
    b2 = d("b2", [D], F32)
    g1, be1, g2, be2 = (d(n, [D], DT) for n in ("g1", "be1", "g2", "be2"))
    ones = d("ones", [512], DT)
    outT = nc.dram_tensor("outT", [D, SH], F32, kind="ExternalOutput").ap()

    with tile.TileContext(nc) as tc:
        _emit(nc, tc, locals())


def _emit(nc, tc, io):
    P = 128
    xT, xsT, maskT = io["xT"], io["xsT"], io["maskT"]
    outT = io["outT"]
    S, D, F, H = io["S"], io["D"], io["F"], io["H"]
    SH, DC, FC, TB, TC, Dh = io["SH"], io["DC"], io["FC"], io["TB"], io["TC"], io["Dh"]
    pack_scores = io["pack_scores"]
    HPC = P // Dh              # heads per 128-feature chunk (2)

    from contextlib import ExitStack
    with ExitStack() as root:
        # ---- global pools (live for the whole kernel) ----
        gconst = root.enter_context(tc.tile_pool(name="gconst", bufs=1))
        gbig = root.enter_context(tc.tile_pool(name="gbig", bufs=1))

        ones_row = gconst.tile([1, P], DT)        # lhsT for K=1 broadcasts
        nc.sync.dma_start(out=ones_row, in_=io["ones"][None, :P])
        ones_col = gconst.tile([P, 1], DT)        # lhsT for partition sums
        nc.sync.dma_start(out=ones_col, in_=io["ones"][:P, None])
        eps_sb = gconst.tile([1, 1], F32)         # LN epsilon as a bias AP
        nc.vector.memset(eps_sb, EPS)



        xs_sb = gbig.tile([P, DC, SH], DT)        # own x shard, transposed (residual)
        nc.sync.dma_start(out=xs_sb, in_=xsT.rearrange("(c p) s -> p c s", p=P))
        xs16_sb = gbig.tile([P, DC, SH], DT16)    # fp16 twin for the q projection
        nc.sync.dma_start(out=xs16_sb, in_=io["xs16"].rearrange("(c p) s -> p c s", p=P))

        def load_vec(name, chunks):               # (n,) -> [128, chunks]
            t = gconst.tile([P, chunks], F32, tag=f"vec_{name}", name=f"vec_{name}")
            nc.sync.dma_start(out=t, in_=io[name].rearrange("(c p) -> p c", p=P))
            return t

        bq_sb, bk_sb, bo_sb = (load_vec(n, DC) for n in ("bq", "bk", "bo"))
        bv_row = gconst.tile([1, D], DT)          # rhs for the +bv outer product
        nc.sync.dma_start(out=bv_row, in_=io["bv"][None, :])
        b1_sb = load_vec("b1", FC)
        b2_sb = load_vec("b2", DC)
        def load_row(name):                       # (n,) -> [1, n] f32r row
            t = gconst.tile([1, D], DT, tag=f"row_{name}", name=f"row_{name}")
            nc.sync.dma_start(out=t, in_=io[name][None, :])
            return t

        g1_row, be1_row, g2_row, be2_row = (load_row(n) for n in ("g1", "be1", "g2", "be2"))
        ones_s = gconst.tile([1, SH], DT)
        nc.sync.dma_start(out=ones_s, in_=io["ones"][None, :SH])

        # PE warmup: dummy matmuls on the first-arriving input keep the HAM
        # activity monitor busy so real matmuls start at 2.4 GHz
        with tc.tile_pool(name="warmps", bufs=1, space="PSUM") as warmps:
            wps = warmps.tile([1, SH], F32)
            for _ in range(8):
                nc.tensor.matmul(wps, lhsT=ones_col, rhs=xs_sb[:, 0, :],
                                 start=True, stop=True)

        attn_sb = gbig.tile([P, DC, SH], DT16)    # normalized attention output^T

        # weight-stream pool opened early so phase-3 DMAs can prefetch
        p3w = root.enter_context(tc.tile_pool(name="p3w", bufs=4))

        # =========== phase 1 then attention (GH=4, pair-batched) ===========
        with tc.tile_pool(name="attn_big", bufs=1) as abig:
            kT_sb = abig.tile([P, DC, S], DT16)       # (x@wk)^T, full sequence
            qT_sb = abig.tile([P, DC, SH], DT16)      # (xs@wq)^T
            v_sb = abig.tile([P, TC, H, Dh + 1], BF16)  # v chunks + ones column
            nc.vector.memset(v_sb[:, :, :, Dh:Dh + 1], 1.0)

            # ---------------- phase 1: qT, kT, v projections ----------------
            with tc.tile_pool(name="p1w", bufs=1) as p1w, \
                 tc.tile_pool(name="p1x", bufs=4) as p1x, \
                 tc.tile_pool(name="p1ps", bufs=6, space="PSUM") as p1ps:

                def load_w(name):                 # (D, n) -> [128, DC, n]
                    w = io[name]
                    t = p1w.tile([P, DC, w.shape[1]], DT16, tag=f"w_{name}", name=f"w_{name}")
                    nc.sync.dma_start(out=t, in_=w.rearrange("(c p) n -> p c n", p=P))
                    return t

                # first t-block of xT starts transferring before anything else
                xt0 = []
                for ci in range(DC):
                    t = p1x.tile([P, 512], DT16, tag=f"xt{ci}", name=f"xt0_{ci}")
                    nc.sync.dma_start(out=t, in_=xT[ds(ci * P, P), ts(0, 512)])
                    xt0.append(t)

                wq_sb, wk_sb, wv_sb = load_w("wq"), load_w("wk"), load_w("wv")

                # broadcast bv across partitions once: bvb[p, do] = bv[do]
                bv_ps = p1ps.tile([P, D], F32, tag="ps")
                nc.tensor.matmul(bv_ps, lhsT=ones_row[:, :P], rhs=bv_row,
                                 start=True, stop=True)
                bvb_sb = p1w.tile([P, D], F32, tag="bvb")
                nc.vector.tensor_copy(bvb_sb, bv_ps)

                # qT[do, s-shard] first
                for co in range(DC):
                    ps = p1ps.tile([P, SH], F32, tag="ps")
                    for ci in range(DC):
                        nc.tensor.matmul(ps, lhsT=wq_sb[:, ci, ds(co * P, P)],
                                         rhs=xs16_sb[:, ci, :], start=(ci == 0), stop=(ci == DC - 1))
                    nc.scalar.activation(out=qT_sb[:, co, :], in_=ps,
                                         func=AF.Identity, bias=bq_sb[:, co:co + 1])

                for tb in range(TB):              # 512-wide t blocks
                    if tb == 0:
                        xt = xt0
                    else:
                        xt = []
                        for ci in range(DC):
                            t = p1x.tile([P, 512], DT16, tag=f"xt{ci}")
                            nc.sync.dma_start(out=t, in_=xT[ds(ci * P, P), ts(tb, 512)])
                            xt.append(t)
                    # kT[do, t-block]
                    for co in range(DC):
                        ps = p1ps.tile([P, 512], F32, tag="ps")
                        for ci in range(DC):
                            nc.tensor.matmul(ps, lhsT=wk_sb[:, ci, ds(co * P, P)],
                                             rhs=xt[ci], start=(ci == 0), stop=(ci == DC - 1))
                        nc.scalar.activation(out=kT_sb[:, co, ts(tb, 512)], in_=ps,
                                             func=AF.Identity, bias=bk_sb[:, co:co + 1])
                    # v[t-block, :] (natural layout) + bv
                    for tj in range(4):
                        ps = p1ps.tile([P, D], F32, tag="ps")
                        for ci in range(DC):
                            nc.tensor.matmul(ps, lhsT=xt[ci][:, ds(tj * P, P)],
                                             rhs=wv_sb[:, ci, :], start=(ci == 0), stop=(ci == DC - 1))
                        nc.vector.tensor_add(
                            out=v_sb[:, tb * 4 + tj, :, 0:Dh],
                            in0=ps.rearrange("p (h d) -> p h d", h=H),
                            in1=bvb_sb.rearrange("p (h d) -> p h d", h=H))

            # ---------------- phase 2: attention ----------------
            GH = 4 if H >= 4 else H
            NPR = GH // HPC
            with tc.tile_pool(name="p2", bufs=3) as p2, \
                 tc.tile_pool(name="p2a", bufs=4) as p2a, \
                 tc.tile_pool(name="p2ps", bufs=1, space="PSUM") as p2ps:
                for g in range(H // GH):
                    heads = list(range(g * GH, (g + 1) * GH))
                    out_ps = {h: p2ps.tile([Dh + 1, SH], F32, tag=f"out{j}", name=f"out_ps{j}")
                              for j, h in enumerate(heads)}
                    for ti in range(TC):
                        m_t = p2.tile([P, SH], BF16, tag="mask")
                        nc.sync.dma_start(out=m_t, in_=maskT[ds(ti * P, P), :])
                        sc = [p2ps.tile([P, HPC, SH], F32, tag=f"sc{pr}", name=f"sc{pr}")
                              for pr in range(NPR)]
                        for j, h in enumerate(heads):
                            pr, half = j // HPC, j % HPC
                            c = h // HPC
                            pslice = ds(half * Dh, Dh)
                            nc.tensor.matmul(
                                sc[pr][:, half, :], lhsT=kT_sb[pslice, c, ds(ti * P, P)],
                                rhs=qT_sb[pslice, c, :], start=True, stop=True,
                                tile_position=(half * Dh, 0) if pack_scores else None)
                        for pr in range(NPR):
                            a_t = p2a.tile([P, HPC, SH], BF16, tag=f"a{pr}", name=f"a{pr}")
                            nc.scalar.activation(out=a_t, in_=sc[pr], func=AF.Exp)
                            am_t = p2a.tile([P, HPC, SH], BF16, tag=f"am{pr}", name=f"am{pr}")
                            for half in range(HPC):
                                nc.vector.tensor_mul(am_t[:, half, :], a_t[:, half, :], m_t)
                            for half in range(HPC):
                                j = pr * HPC + half
                                h = heads[j]
                                nc.tensor.matmul(out_ps[h], lhsT=v_sb[:, ti, h, :],
                                                 rhs=am_t[:, half, :],
                                                 start=(ti == 0), stop=(ti == TC - 1))
                    for j, h in enumerate(heads):
                        c, half = h // HPC, h % HPC
                        rec = p2.tile([1, SH], DT, tag="rec")
                        with nc.allow_low_precision(reason="fp32 storage"):
                            nc.vector.reciprocal(rec, out_ps[h][Dh:Dh + 1, :])
                        bc = p2ps.tile([Dh, SH], F32, tag=f"sc{j // HPC}", name=f"bc{j}")
                        nc.tensor.matmul(bc, lhsT=ones_row[:1, :Dh], rhs=rec,
                                         start=True, stop=True)
                        bc_sb = p2.tile([Dh, SH], DT, tag="bcsb")
                        nc.scalar.copy(bc_sb, bc)
                        nc.vector.tensor_mul(attn_sb[ds(half * Dh, Dh), c, :],
                                             out_ps[h][0:Dh, :], bc_sb)

        # ---------------- phase 3: out proj + LN1 + FFN + LN2 ----------------
        with tc.tile_pool(name="p3", bufs=2) as p3, \
             tc.tile_pool(name="p3big", bufs=1) as p3big, \
             tc.tile_pool(name="p3ps", bufs=2, space="PSUM") as p3ps, \
             tc.tile_pool(name="p3st", bufs=1, space="PSUM") as p3st, \
             tc.tile_pool(name="p3bc", bufs=2, space="PSUM") as p3bc:

            def layernorm(src, g_row, be_row, dst):
                """src/dst: lists of DC [128, SH] tiles; stats over partitions."""
                mu_ps = p3st.tile([1, SH], F32, tag="mu")
                m2_ps = p3st.tile([1, SH], F32, tag="m2")
                for c in range(DC):
                    nc.tensor.matmul(mu_ps, lhsT=ones_col, rhs=src[c],
                                     start=(c == 0), stop=(c == DC - 1))
                for c in range(DC):
                    sq = p3.tile([P, SH], DT, tag="sq")
                    nc.scalar.activation(out=sq, in_=src[c], func=AF.Square)
                    nc.tensor.matmul(m2_ps, lhsT=ones_col, rhs=sq,
                                     start=(c == 0), stop=(c == DC - 1))
                mu_s = p3.tile([1, SH], DT, tag="mu_s")
                m2_s = p3.tile([1, SH], DT, tag="m2_s")
                nc.vector.tensor_scalar_mul(mu_s, mu_ps, -1.0 / D)  # negated mean
                nc.vector.tensor_scalar_mul(m2_s, m2_ps, 1.0 / D)
                var_s = p3.tile([1, SH], DT, tag="var_s")
                nc.vector.tensor_mul(var_s, mu_s, mu_s)
                nc.vector.tensor_sub(var_s, m2_s, var_s)
                # rstd = (var+eps)^-1/2 = exp(-0.5 ln(var+eps)) on ACT
                rstd_s = p3.tile([1, SH], DT, tag="rstd_s")
                sd_s = p3.tile([1, SH], DT, tag="sd_s")
                nc.scalar.activation(out=sd_s, in_=var_s, func=AF.Sqrt, bias=eps_sb)
                with nc.allow_low_precision(reason="fp32 storage"):
                    nc.vector.reciprocal(rstd_s, sd_s)
                # offset row: -mu*rstd
                off_s = p3.tile([1, SH], DT, tag="off_s")
                nc.vector.tensor_mul(off_s, mu_s, rstd_s)
                # per-chunk fused affine: dst = x*(g x rstd) + (g x (-mu rstd) + be x 1)
                for c in range(DC):
                    sc_b = p3bc.tile([P, SH], F32, tag="sc_b")
                    of_b = p3bc.tile([P, SH], F32, tag="of_b")
                    nc.tensor.matmul(sc_b, lhsT=g_row[:, ds(c * P, P)], rhs=rstd_s,
                                     start=True, stop=True)
                    nc.tensor.matmul(of_b, lhsT=g_row[:, ds(c * P, P)], rhs=off_s,
                                     start=True, stop=False)
                    nc.tensor.matmul(of_b, lhsT=be_row[:, ds(c * P, P)], rhs=ones_s,
                                     start=False, stop=True)
                    t = p3.tile([P, SH], DT, tag="lnt")
                    nc.vector.tensor_mul(t, src[c], sc_b)
                    nc.vector.tensor_add(dst[c], t, of_b)

            # out projection + residual -> xr; bo is pre-folded into the
            # residual operand so the per-chunk join is a single DVE add
            wo_v = io["wo"].rearrange("(c p) n -> p c n", p=P)
            xr = [p3big.tile([P, SH], DT, tag=f"xr{c}", name=f"xr{c}") for c in range(DC)]
            xsb = [p3big.tile([P, SH], DT, tag=f"xsb{c}", name=f"xsb{c}") for c in range(DC)]
            for c in range(DC):
                nc.vector.tensor_scalar_add(xsb[c], xs_sb[:, c, :], bo_sb[:, c:c + 1])
            for co in range(DC):
                ps = p3ps.tile([P, SH], F32, tag="ps")
                wt = p3w.tile([P, DC, P], DT16, tag="wt")
                nc.sync.dma_start(out=wt, in_=wo_v[:, :, ds(co * P, P)])
                for ci in range(DC):
                    nc.tensor.matmul(ps, lhsT=wt[:, ci, :], rhs=attn_sb[:, ci, :],
                                     start=(ci == 0), stop=(ci == DC - 1))
                nc.vector.tensor_add(xr[co], ps, xsb[co])

            x1 = [p3big.tile([P, SH], DT, tag=f"x1{c}", name=f"x1{c}") for c in range(DC)]
            layernorm(xr, g1_row, be1_row, x1)
            x1h = [p3big.tile([P, SH], DT16, tag=f"x1h{c}", name=f"x1h{c}") for c in range(DC)]
            for c in range(DC):
                nc.scalar.copy(x1h[c], x1[c])

            # FFN
            w1_v = io["w1"].rearrange("(c p) n -> p c n", p=P)
            w2_v = io["w2"].rearrange("(c p) n -> p c n", p=P)
            hT = p3big.tile([P, FC, SH], DT16, tag="hT")
            for fc in range(FC):
                ps = p3ps.tile([P, SH], F32, tag="ps")
                wt = p3w.tile([P, DC, P], DT16, tag="wt")
                nc.sync.dma_start(out=wt, in_=w1_v[:, :, ds(fc * P, P)])
                for ci in range(DC):
                    nc.tensor.matmul(ps, lhsT=wt[:, ci, :], rhs=x1h[ci],
                                     start=(ci == 0), stop=(ci == DC - 1))
                nc.scalar.activation(out=hT[:, fc, :], in_=ps, func=AF.Relu,
                                     bias=b1_sb[:, fc:fc + 1])
            xr2 = [p3big.tile([P, SH], DT, tag=f"xr2{c}", name=f"xr2{c}") for c in range(DC)]
            x1b = [p3big.tile([P, SH], DT, tag=f"x1b{c}", name=f"x1b{c}") for c in range(DC)]
            for c in range(DC):
                nc.vector.tensor_scalar_add(x1b[c], x1[c], b2_sb[:, c:c + 1])
            for co in range(DC):
                ps = p3ps.tile([P, SH], F32, tag="ps")
                wt2 = p3w.tile([P, FC, P], DT16, tag="wt2")
                nc.sync.dma_start(out=wt2, in_=w2_v[:, :, ds(co * P, P)])
                for fc in range(FC):
                    nc.tensor.matmul(ps, lhsT=wt2[:, fc, :], rhs=hT[:, fc, :],
                                     start=(fc == 0), stop=(fc == FC - 1))
                nc.vector.tensor_add(xr2[co], ps, x1b[co])

            x2 = [p3big.tile([P, SH], F32, tag=f"x2{c}", name=f"x2{c}") for c in range(DC)]
            layernorm(xr2, g2_row, be2_row, x2)
            for c in range(DC):
                nc.sync.dma_start(out=outT[ds(c * P, P), :], in_=x2[c])


# ---------------------------------------------------------------------------
# host-side entry point
# ---------------------------------------------------------------------------

_CACHE = {}


def _get_compiled(S, D, F, H):
    key = (S, D, F, H)
    if key not in _CACHE:
        nc = bacc.Bacc("TRN2", target_bir_lowering=False, debug=False,
                       num_devices=N_CORES)
        build_encoder_kernel(nc, S=S, D=D, F=F, H=H, n_cores=N_CORES)
        nc.compile()
        _CACHE[key] = nc
    return _CACHE[key]


def make_in_maps(x, mask, weights, S, D, n_cores=N_CORES):
    """Shard + lay out inputs per core. x: (S, D) f32; mask: (S, S) int."""
    SH = S // n_cores
    xT = np.ascontiguousarray(x.T)                       # (D, S)
    maskb = (mask != 0)
    in_maps = []
    for c in range(n_cores):
        sl = slice(c * SH, (c + 1) * SH)
        im = {
            "xT": xT.astype(np.float16),
            "xsT": np.ascontiguousarray(xT[:, sl]),
            "xs16": np.ascontiguousarray(xT[:, sl]).astype(np.float16),
            "maskT": np.ascontiguousarray(maskb[sl, :].T).astype(ml_dtypes.bfloat16),
            "ones": np.ones(512, np.float32),
        }
        im.update({k: (v.astype(np.float16) if k in ("wq", "wk", "wv", "wo", "w1", "w2")
                       else v) for k, v in weights.items()})
        in_maps.append(im)
    return in_maps


def kernel(**inputs):
    x = np.asarray(inputs["x"], np.float32)
    mask = np.asarray(inputs["mask"])
    B, S, D = x.shape
    F = inputs["w1"].shape[1]
    H = 8
    assert B == 1
    weights = {k: np.asarray(inputs[k], np.float32)
               for k in ("wq", "wk", "wv", "wo", "w1", "w2",
                         "bq", "bk", "bv", "bo", "b1", "b2",
                         "g1", "be1", "g2", "be2")}
    nc = _get_compiled(S, D, F, H)
    in_maps = make_in_maps(x[0], mask, weights, S, D)
    res = run_bass_kernel_spmd(nc, in_maps, list(range(N_CORES)))
    SH = S // N_CORES
    out = np.empty((S, D), np.float32)
    for c in range(N_CORES):
        out[c * SH:(c + 1) * SH, :] = res.results[c]["outT"].T
    return out[None]
